# revision 11
# baseline (speedup 1.0000x reference)
"""DetectionLoss Trainium kernel.

Problem: B=32 images, N=300 predictions, M=50 ground truths, D=2 dims.
Reference semantics: per-image Hungarian matching (host-side, float64,
no_grad — exactly as in the reference), then three losses:
  loss_pos   = LAMBDA_POS  * sum_b mean_{m,d} |pred_c[b,pi[b,m]] - gt_c[b,gi[b,m]]| / B
  loss_obj   = LAMBDA_CONF * sum_b mean_m softplus(-conf[b,pi[b,m]]) / B
  loss_noobj = LAMBDA_NOOBJ* sum_b (sum_{unmatched n} softplus(conf[b,n]))/(N-M) / B

Sharding: data-parallel over batch, 4 images per core on 8 cores.

Device kernel design: the host computes the matching (as the reference
does) and packs, per image, one 500-float row:
  [ conf at matched preds (50) | conf at unmatched preds (250)
    | matched pred centroids (100) | matched gt centroids (100) ]
Each core gets a [4, 500] f32 tile (one image per partition) and computes
three per-image sums with fused activation+accumulate ops:
  col0 = sum softplus(-conf_matched)      (ACT Softplus, scale=-1, accum)
  col1 = sum softplus(conf_unmatched)     (ACT Softplus, accum)
  col2 = sum |mp - mg|                    (DVE subtract, ACT Abs, accum)
The host applies the lambda / B / mean scalings and sums the 32 per-image
partials (the "all-reduce" of the three scalar sums).
"""

import numpy as np

COST_POS = 5.0
COST_CONF = 1.0
LAMBDA_POS = 5.0
LAMBDA_CONF = 2.0
LAMBDA_NOOBJ = 0.5

B, N, M, D = 32, 300, 50, 2
N_CORES = 8
PER_CORE = B // N_CORES  # 4
ROW = M + (N - M) + M * D + M * D + 1  # 501 (last col = 1.0, Ln bias)


# ---------------------------------------------------------------------------
# Host-side Hungarian matching — verbatim float64 numpy port of the
# reference (scipy-equivalent Jonker-Volgenant), so the matched indices are
# bit-identical to the reference's.
# ---------------------------------------------------------------------------

def _lsa(cost):
    cost = np.asarray(cost, dtype=np.float64)
    transposed = cost.shape[1] < cost.shape[0]
    if transposed:
        cost = cost.T
    nr, nc = cost.shape  # nr <= nc
    u = np.zeros(nr)
    v = np.zeros(nc)
    col4row = np.full(nr, -1, dtype=np.int64)
    row4col = np.full(nc, -1, dtype=np.int64)
    for cur_row in range(nr):
        shortest = np.full(nc, np.inf)
        pathback = np.full(nc, -1, dtype=np.int64)
        SR = np.zeros(nr, dtype=bool)
        SC = np.zeros(nc, dtype=bool)
        remaining = np.ones(nc, dtype=bool)
        min_val = 0.0
        i = cur_row
        sink = -1
        while sink == -1:
            SR[i] = True
            rem = np.flatnonzero(remaining)
            new_cost = min_val + cost[i, rem] - u[i] - v[rem]
            better = new_cost < shortest[rem]
            idx = rem[better]
            shortest[idx] = new_cost[better]
            pathback[idx] = i
            j = rem[np.argmin(shortest[rem])]
            min_val = shortest[j]
            remaining[j] = False
            SC[j] = True
            if row4col[j] == -1:
                sink = j
            else:
                i = row4col[j]
        u[cur_row] += min_val
        rows = np.flatnonzero(SR)
        rows = rows[rows != cur_row]
        u[rows] += min_val - shortest[col4row[rows]]
        v[SC] -= min_val - shortest[SC]
        j = sink
        while True:
            i = pathback[j]
            row4col[j] = i
            col4row[i], j = j, col4row[i]
            if i == cur_row:
                break
    if transposed:
        rows = col4row
        cols = np.arange(nr)
        order = np.argsort(rows)
        return rows[order], cols[order]
    return np.arange(nr), col4row


def _match(pred_c, conf, gt_c):
    Bs, _, _ = pred_c.shape
    Ms = gt_c.shape[1]
    pred_idx = np.empty((Bs, Ms), dtype=np.int64)
    gt_idx = np.empty((Bs, Ms), dtype=np.int64)
    for b in range(Bs):
        c_pos = np.abs(pred_c[b][:, None, :] - gt_c[b][None, :, :]).sum(-1)
        c_conf = -1.0 / (1.0 + np.exp(-conf[b]))
        cost = COST_POS * c_pos + COST_CONF * c_conf[:, None]
        r, c = _lsa(cost)
        pred_idx[b] = r
        gt_idx[b] = c
    return pred_idx, gt_idx


# ---------------------------------------------------------------------------
# Bass device kernel (built once, cached)
# ---------------------------------------------------------------------------

_NC = {}


def _build_nc(repeat=1):
    """Build the Bass module. repeat>1 emits the body K times back-to-back
    (serialized through monotonically increasing sems) for slope timing."""
    if repeat in _NC:
        return _NC[repeat]
    import contextlib

    import concourse.bass as bass
    import concourse.mybir as mybir

    f32 = mybir.dt.float32
    nc = bass.Bass("TRN2", target_bir_lowering=False, debug=False,
                   num_devices=N_CORES)
    x = nc.dram_tensor("x", [PER_CORE, ROW], f32, kind="ExternalInput").ap()
    out = nc.dram_tensor("out", [PER_CORE, 3], f32, kind="ExternalOutput").ap()

    # softplus(x) = ln(exp(x)*1 + 1): exp and ln share one ACT table set
    # ("natural_log_exp_and_others"; the Softplus func has no table set in
    # this toolchain). The host pre-negates matched conf, so a single exp
    # pass over all 300 conf values serves both loss_obj and loss_noobj.
    # Raw bass (no TileContext): the dependency chain is linear, and Tile's
    # tail drain needs more sem-wait slots than the CTRL instruction has.
    AF = mybir.ActivationFunctionType
    with contextlib.ExitStack() as ctx:
        block = ctx.enter_context(nc.Block())
        dma_sem = ctx.enter_context(nc.semaphore("dma_sem"))
        dve_sem = ctx.enter_context(nc.semaphore("dve_sem"))
        act_sem = ctx.enter_context(nc.semaphore("act_sem"))
        t = ctx.enter_context(nc.sbuf_tensor([PER_CORE, ROW], f32))
        e = ctx.enter_context(nc.sbuf_tensor([PER_CORE, N], f32))
        diff = ctx.enter_context(nc.sbuf_tensor([PER_CORE, M * D], f32))
        scr = ctx.enter_context(nc.sbuf_tensor([PER_CORE, N - M], f32))
        res = ctx.enter_context(nc.sbuf_tensor([PER_CORE, 3], f32))

        @block.vector
        def _(vector: bass.BassEngine):
            for k in range(repeat):
                vector.wait_ge(dma_sem, 32 * k + 16)
                vector.tensor_sub(diff[:], t[:, 300:400],
                                  t[:, 400:500]).then_inc(dve_sem, 1)

        @block.scalar
        def _(scalar: bass.BassEngine):
            one = t[:, 500:501]
            for k in range(repeat):
                scalar.wait_ge(dma_sem, 32 * k + 16)
                scalar.activation(e[:], t[:, 0:N], AF.Exp)
                scalar.activation(scr[:, 0:M], e[:, 0:M], AF.Ln,
                                  bias=one, accum_out=res[:, 0:1])
                scalar.activation(scr[:, 0:N - M], e[:, M:N], AF.Ln,
                                  bias=one, accum_out=res[:, 1:2])
                scalar.wait_ge(dve_sem, k + 1)
                scalar.activation(scr[:, 0:M * D], diff[:], AF.Abs,
                                  accum_out=res[:, 2:3]).then_inc(act_sem, 1)

        @block.sync
        def _(sync: bass.BassEngine):
            for k in range(repeat):
                if k > 0:
                    # prior iteration fully drained (its out-DMA read res)
                    sync.wait_ge(dma_sem, 32 * k)
                sync.dma_start(out=t[:], in_=x[:]).then_inc(dma_sem, 16)
                sync.wait_ge(act_sem, k + 1)
                sync.dma_start(out=out[:], in_=res[:]).then_inc(dma_sem, 16)
            sync.wait_ge(dma_sem, 32 * repeat)

    _NC[repeat] = nc
    return nc


def _run_device(pack, trace=False, **kw):
    """pack: [B, ROW] f32 → per-image sums [B, 3] (obj, noobj, pos)."""
    from concourse.bass_utils import run_bass_kernel_spmd

    nc = _build_nc()
    in_maps = [{"x": pack[c * PER_CORE:(c + 1) * PER_CORE]}
               for c in range(N_CORES)]
    r = run_bass_kernel_spmd(nc, in_maps, list(range(N_CORES)),
                             trace=trace, **kw)
    sums = np.concatenate([r.results[c]["out"] for c in range(N_CORES)],
                          axis=0)
    return sums, r


def _pack_inputs(pred_centroids, pred_conf, gt_centroids):
    pred_idx, gt_idx = _match(
        np.asarray(pred_centroids, np.float64),
        np.asarray(pred_conf, np.float64),
        np.asarray(gt_centroids, np.float64),
    )
    bidx = np.arange(B)[:, None]
    conf_m = -pred_conf[bidx, pred_idx]                    # [B, M], negated
    um = np.ones((B, N), dtype=bool)
    um[bidx, pred_idx] = False
    conf_u = pred_conf[um].reshape(B, N - M)               # [B, N-M]
    mp = pred_centroids[bidx, pred_idx].reshape(B, M * D)  # [B, 100]
    mg = gt_centroids[bidx, gt_idx].reshape(B, M * D)      # [B, 100]
    ones = np.ones((B, 1), np.float32)                     # Ln bias column
    return np.ascontiguousarray(
        np.concatenate([conf_m, conf_u, mp, mg, ones],
                       axis=1).astype(np.float32))


def measure_hw_ns(pack, k_long=101, reps=10):
    """Steady-state per-iteration device time: build a NEFF whose body runs
    the kernel K times back-to-back (fully serialized through sems), time
    K=1 and K=k_long end-to-end through the standard run path (best of
    `reps`, so the ms-scale axon RPC overhead cancels in the subtraction),
    and report slope (t_K - t_1)/(K - 1)."""
    import time

    def timed(k):
        nc = _build_nc(repeat=k)
        in_maps = [{"x": pack[c * PER_CORE:(c + 1) * PER_CORE]}
                   for c in range(N_CORES)]
        from concourse.bass_utils import run_bass_kernel_spmd
        run_bass_kernel_spmd(nc, in_maps, list(range(N_CORES)))  # compile+warm
        best = float("inf")
        for _ in range(reps):
            t0 = time.perf_counter()
            run_bass_kernel_spmd(nc, in_maps, list(range(N_CORES)))
            best = min(best, time.perf_counter() - t0)
        return best

    t1 = timed(1)
    tk = timed(k_long)
    ns = (tk - t1) / (k_long - 1) * 1e9
    print(f"  [measure] t1={t1*1e6:.1f} us  t{k_long}={tk*1e6:.1f} us  "
          f"slope={ns:.0f} ns/iter")
    return ns


def kernel(pred_centroids, pred_logits, pred_conf, gt_centroids, gt_classes):
    pred_centroids = np.asarray(pred_centroids, np.float32)
    pred_conf = np.asarray(pred_conf, np.float32)
    gt_centroids = np.asarray(gt_centroids, np.float32)

    pack = _pack_inputs(pred_centroids, pred_conf, gt_centroids)
    sums, _ = _run_device(pack)

    obj = sums[:, 0].astype(np.float64)
    noobj = sums[:, 1].astype(np.float64)
    pos = sums[:, 2].astype(np.float64)
    loss_pos = np.float32(LAMBDA_POS * (pos / (M * D)).sum() / B)
    loss_conf_obj = np.float32(LAMBDA_CONF * (obj / M).sum() / B)
    loss_conf_noobj = np.float32(LAMBDA_NOOBJ * (noobj / (N - M)).sum() / B)
    loss_total = np.float32(loss_pos + loss_conf_obj + loss_conf_noobj)
    n_matched = np.float32(M)
    return loss_pos, loss_conf_obj, loss_conf_noobj, loss_total, n_matched


# revision 20
# speedup vs baseline: 1.3003x; 1.3003x over previous
"""DetectionLoss Trainium kernel.

Problem: B=32 images, N=300 predictions, M=50 ground truths, D=2 dims.
Reference semantics: per-image Hungarian matching (host-side, float64,
no_grad — exactly as in the reference), then three losses:
  loss_pos   = LAMBDA_POS  * sum_b mean_{m,d} |pred_c[b,pi[b,m]] - gt_c[b,gi[b,m]]| / B
  loss_obj   = LAMBDA_CONF * sum_b mean_m softplus(-conf[b,pi[b,m]]) / B
  loss_noobj = LAMBDA_NOOBJ* sum_b (sum_{unmatched n} softplus(conf[b,n]))/(N-M) / B

Sharding: data-parallel over batch, 4 images per core on 8 cores.

Device kernel design: the host computes the matching (as the reference
does) and packs, per image, one 500-float row:
  [ conf at matched preds (50) | conf at unmatched preds (250)
    | matched pred centroids (100) | matched gt centroids (100) ]
Each core gets a [4, 500] f32 tile (one image per partition) and computes
three per-image sums with fused activation+accumulate ops:
  col0 = sum softplus(-conf_matched)      (ACT Softplus, scale=-1, accum)
  col1 = sum softplus(conf_unmatched)     (ACT Softplus, accum)
  col2 = sum |mp - mg|                    (DVE subtract, ACT Abs, accum)
The host applies the lambda / B / mean scalings and sums the 32 per-image
partials (the "all-reduce" of the three scalar sums).
"""

import numpy as np

COST_POS = 5.0
COST_CONF = 1.0
LAMBDA_POS = 5.0
LAMBDA_CONF = 2.0
LAMBDA_NOOBJ = 0.5

B, N, M, D = 32, 300, 50, 2
N_CORES = 8
PER_CORE = B // N_CORES  # 4

# Device data layout, per core: [80, 51] f32. Engine APs must start at a
# 32-aligned partition, so the layout only ever slices partitions at 0/64:
#   rows 0:8,   cols 0:25 = -conf[matched]   (2 rows x 25 per image)
#   rows 8:48,  cols 0:25 =  conf[unmatched] (10 rows x 25 per image)
#   rows 48:64            =  zero pad (lets the ln accum span 0:64 and the
#                            abs accum start at 64)
#   rows 64:80, cols 0:25 =  matched pred centroids (4 rows x 25 per image)
#   rows 64:80, cols 25:50 = matched gt centroids  (same partitions, so the
#                            DVE subtract slices columns, not partitions)
#   col 50 = 1.0 (per-partition Ln bias)
# One exp + one ln(.+1)-with-accum pass over rows 0:64 gives all per-row
# softplus sums; |mp-mg| accum fills rows 64:80. Host regroups per image.
P_ROWS = 80
COLS = 51
CHUNK = 25


# ---------------------------------------------------------------------------
# Host-side Hungarian matching — verbatim float64 numpy port of the
# reference (scipy-equivalent Jonker-Volgenant), so the matched indices are
# bit-identical to the reference's.
# ---------------------------------------------------------------------------

def _lsa(cost):
    cost = np.asarray(cost, dtype=np.float64)
    transposed = cost.shape[1] < cost.shape[0]
    if transposed:
        cost = cost.T
    nr, nc = cost.shape  # nr <= nc
    u = np.zeros(nr)
    v = np.zeros(nc)
    col4row = np.full(nr, -1, dtype=np.int64)
    row4col = np.full(nc, -1, dtype=np.int64)
    for cur_row in range(nr):
        shortest = np.full(nc, np.inf)
        pathback = np.full(nc, -1, dtype=np.int64)
        SR = np.zeros(nr, dtype=bool)
        SC = np.zeros(nc, dtype=bool)
        remaining = np.ones(nc, dtype=bool)
        min_val = 0.0
        i = cur_row
        sink = -1
        while sink == -1:
            SR[i] = True
            rem = np.flatnonzero(remaining)
            new_cost = min_val + cost[i, rem] - u[i] - v[rem]
            better = new_cost < shortest[rem]
            idx = rem[better]
            shortest[idx] = new_cost[better]
            pathback[idx] = i
            j = rem[np.argmin(shortest[rem])]
            min_val = shortest[j]
            remaining[j] = False
            SC[j] = True
            if row4col[j] == -1:
                sink = j
            else:
                i = row4col[j]
        u[cur_row] += min_val
        rows = np.flatnonzero(SR)
        rows = rows[rows != cur_row]
        u[rows] += min_val - shortest[col4row[rows]]
        v[SC] -= min_val - shortest[SC]
        j = sink
        while True:
            i = pathback[j]
            row4col[j] = i
            col4row[i], j = j, col4row[i]
            if i == cur_row:
                break
    if transposed:
        rows = col4row
        cols = np.arange(nr)
        order = np.argsort(rows)
        return rows[order], cols[order]
    return np.arange(nr), col4row


def _match(pred_c, conf, gt_c):
    Bs, _, _ = pred_c.shape
    Ms = gt_c.shape[1]
    pred_idx = np.empty((Bs, Ms), dtype=np.int64)
    gt_idx = np.empty((Bs, Ms), dtype=np.int64)
    for b in range(Bs):
        c_pos = np.abs(pred_c[b][:, None, :] - gt_c[b][None, :, :]).sum(-1)
        c_conf = -1.0 / (1.0 + np.exp(-conf[b]))
        cost = COST_POS * c_pos + COST_CONF * c_conf[:, None]
        r, c = _lsa(cost)
        pred_idx[b] = r
        gt_idx[b] = c
    return pred_idx, gt_idx


# ---------------------------------------------------------------------------
# Bass device kernel (built once, cached)
# ---------------------------------------------------------------------------

_NC = {}


def _build_nc(repeat=1):
    """Build the Bass module. repeat>1 emits the body K times back-to-back
    (serialized through monotonically increasing sems) for slope timing."""
    if repeat in _NC:
        return _NC[repeat]
    import contextlib

    import concourse.bass as bass
    import concourse.mybir as mybir

    f32 = mybir.dt.float32
    nc = bass.Bass("TRN2", target_bir_lowering=False, debug=False,
                   num_devices=N_CORES)
    x = nc.dram_tensor("x", [P_ROWS, COLS], f32, kind="ExternalInput").ap()
    out = nc.dram_tensor("out", [P_ROWS, 1], f32, kind="ExternalOutput").ap()

    # softplus(x) = ln(exp(x)*1 + 1): exp and ln share one ACT table set
    # ("natural_log_exp_and_others"; the Softplus func has no table set in
    # this toolchain). The host pre-negates matched conf, so a single exp
    # pass over rows 0:48 serves both loss_obj and loss_noobj, and the
    # accum_out per-partition sums separate the two loss groups by row.
    # Raw bass (no TileContext): the dependency chain is linear, and Tile's
    # tail drain needs more sem-wait slots than the CTRL instruction has.
    # The out-DMA is issued by the ACT engine itself (program order after
    # the last accum) so no act->SP semaphore hop is on the critical path.
    AF = mybir.ActivationFunctionType
    with contextlib.ExitStack() as ctx:
        block = ctx.enter_context(nc.Block(no_gpsimd_drain=True))
        dma_sem = ctx.enter_context(nc.semaphore("dma_sem"))
        dve_sem = ctx.enter_context(nc.semaphore("dve_sem"))
        t = ctx.enter_context(nc.sbuf_tensor([P_ROWS, COLS], f32))
        e = ctx.enter_context(nc.sbuf_tensor([64, CHUNK], f32))
        diff = ctx.enter_context(nc.sbuf_tensor([16, CHUNK], f32))
        scr = ctx.enter_context(nc.sbuf_tensor([64, CHUNK], f32))
        res = ctx.enter_context(nc.sbuf_tensor([P_ROWS, 1], f32))

        @block.vector
        def _(vector: bass.BassEngine):
            for k in range(repeat):
                vector.wait_ge(dma_sem, 32 * k + 16)
                vector.tensor_sub(diff[:], t[64:80, 0:CHUNK],
                                  t[64:80, CHUNK:2 * CHUNK]).then_inc(
                    dve_sem, 1)

        @block.scalar
        def _(scalar: bass.BassEngine):
            one = t[0:64, 50:51]
            for k in range(repeat):
                scalar.wait_ge(dma_sem, 32 * k + 16)
                scalar.activation(e[:], t[0:64, 0:CHUNK], AF.Exp)
                scalar.activation(scr[:], e[:], AF.Ln, bias=one,
                                  accum_out=res[0:64, 0:1])
                scalar.wait_ge(dve_sem, k + 1)
                scalar.activation(scr[0:16, :], diff[:], AF.Abs,
                                  accum_out=res[64:80, 0:1])
                scalar.dma_start(out=out[:], in_=res[:]).then_inc(dma_sem, 16)

        @block.sync
        def _(sync: bass.BassEngine):
            for k in range(repeat):
                if k > 0:
                    # prior iteration fully drained (its out-DMA read res)
                    sync.wait_ge(dma_sem, 32 * k)
                sync.dma_start(out=t[:], in_=x[:]).then_inc(dma_sem, 16)
            sync.wait_ge(dma_sem, 32 * repeat)

    _NC[repeat] = nc
    return nc


def _run_device(pack, trace=False, **kw):
    """pack: [N_CORES, 80, 26] f32 → per-image sums [B, 3] (obj, noobj, pos)."""
    from concourse.bass_utils import run_bass_kernel_spmd

    nc = _build_nc()
    in_maps = [{"x": pack[c]} for c in range(N_CORES)]
    r = run_bass_kernel_spmd(nc, in_maps, list(range(N_CORES)),
                             trace=trace, **kw)
    sums = np.empty((B, 3), np.float64)
    for c in range(N_CORES):
        v = r.results[c]["out"].reshape(P_ROWS).astype(np.float64)
        sums[c * PER_CORE:(c + 1) * PER_CORE, 0] = \
            v[0:8].reshape(PER_CORE, 2).sum(1)
        sums[c * PER_CORE:(c + 1) * PER_CORE, 1] = \
            v[8:48].reshape(PER_CORE, 10).sum(1)
        sums[c * PER_CORE:(c + 1) * PER_CORE, 2] = \
            v[64:80].reshape(PER_CORE, 4).sum(1)
    return sums, r


def _pack_inputs(pred_centroids, pred_conf, gt_centroids):
    pred_idx, gt_idx = _match(
        np.asarray(pred_centroids, np.float64),
        np.asarray(pred_conf, np.float64),
        np.asarray(gt_centroids, np.float64),
    )
    bidx = np.arange(B)[:, None]
    conf_m = -pred_conf[bidx, pred_idx]                    # [B, M], negated
    um = np.ones((B, N), dtype=bool)
    um[bidx, pred_idx] = False
    conf_u = pred_conf[um].reshape(B, N - M)               # [B, N-M]
    mp = pred_centroids[bidx, pred_idx].reshape(B, M * D)  # [B, 100]
    mg = gt_centroids[bidx, gt_idx].reshape(B, M * D)      # [B, 100]
    X = np.zeros((N_CORES, P_ROWS, COLS), np.float32)
    X[:, :, 50] = 1.0                                      # Ln bias column
    X[:, 0:8, 0:CHUNK] = conf_m.reshape(N_CORES, 8, CHUNK)
    X[:, 8:48, 0:CHUNK] = conf_u.reshape(N_CORES, 40, CHUNK)
    X[:, 64:80, 0:CHUNK] = mp.reshape(N_CORES, 16, CHUNK)
    X[:, 64:80, CHUNK:2 * CHUNK] = mg.reshape(N_CORES, 16, CHUNK)
    return X


def measure_hw_ns(pack, k_long=101, reps=10):
    """Steady-state per-iteration device time: build a NEFF whose body runs
    the kernel K times back-to-back (fully serialized through sems), time
    K=1 and K=k_long end-to-end through the standard run path (best of
    `reps`, so the ms-scale axon RPC overhead cancels in the subtraction),
    and report slope (t_K - t_1)/(K - 1)."""
    import time

    def timed(k):
        nc = _build_nc(repeat=k)
        in_maps = [{"x": pack[c]} for c in range(N_CORES)]
        from concourse.bass_utils import run_bass_kernel_spmd
        run_bass_kernel_spmd(nc, in_maps, list(range(N_CORES)))  # compile+warm
        best = float("inf")
        for _ in range(reps):
            t0 = time.perf_counter()
            run_bass_kernel_spmd(nc, in_maps, list(range(N_CORES)))
            best = min(best, time.perf_counter() - t0)
        return best

    t1 = timed(1)
    tk = timed(k_long)
    ns = (tk - t1) / (k_long - 1) * 1e9
    print(f"  [measure] t1={t1*1e6:.1f} us  t{k_long}={tk*1e6:.1f} us  "
          f"slope={ns:.0f} ns/iter")
    return ns


def kernel(pred_centroids, pred_logits, pred_conf, gt_centroids, gt_classes):
    pred_centroids = np.asarray(pred_centroids, np.float32)
    pred_conf = np.asarray(pred_conf, np.float32)
    gt_centroids = np.asarray(gt_centroids, np.float32)

    pack = _pack_inputs(pred_centroids, pred_conf, gt_centroids)
    sums, _ = _run_device(pack)

    obj = sums[:, 0].astype(np.float64)
    noobj = sums[:, 1].astype(np.float64)
    pos = sums[:, 2].astype(np.float64)
    loss_pos = np.float32(LAMBDA_POS * (pos / (M * D)).sum() / B)
    loss_conf_obj = np.float32(LAMBDA_CONF * (obj / M).sum() / B)
    loss_conf_noobj = np.float32(LAMBDA_NOOBJ * (noobj / (N - M)).sum() / B)
    loss_total = np.float32(loss_pos + loss_conf_obj + loss_conf_noobj)
    n_matched = np.float32(M)
    return loss_pos, loss_conf_obj, loss_conf_noobj, loss_total, n_matched
